# revision 2
# baseline (speedup 1.0000x reference)
"""Causal attention on 8 TRN2 cores — key-sharded variant.

2 cores per batch; the PAIR splits the KEYS (not queries): parity c owns
key blocks S_c = sorted({2p+c} u {31-2p-c}), each core projects K^T/V for
only its 2048 keys (no duplication) but Q^T for all 4096 queries, then
computes partial unnormalized attention over its keyset for every query
block. Host combines y = (num0+num1)/(ell0+ell1) — exact, since exp uses
no max shift (scores bounded) so both cores use the same shift of 0.
"""

import math
import sys

sys.path.insert(0, "/opt/trn_rl_repo")

import ml_dtypes
import numpy as np

import concourse.mybir as mybir
import concourse.tile as tile
from concourse import bacc
from concourse.bass_utils import run_bass_kernel_spmd
from concourse.masks import make_identity

B = 4
S = 4096
D = 1024
P = 128
DC = D // P
NKB = 32             # global key blocks per batch
NQB = 32             # q blocks per core (all of them)
HALF = S // 2        # keys owned per core
BF16 = mybir.dt.bfloat16
F32 = mybir.dt.float32
NEG = -1.0e9
SCALE = 1.0 / math.sqrt(D)


def _keyset(c):
    return sorted({2 * p + c for p in range(8)} | {31 - 2 * p - c for p in range(8)})


def _L(g):
    return max((g + 2) // 2, 1)  # ceil((g+1)/2), min 1


_LS = [_L(g) for g in range(NQB)]

# attention block order: interleave big and small blocks so a big block's
# long matmul stream hides the small block's serial latency chain
_GORDER = []
_lo, _hi = 0, NQB - 1
while _lo <= _hi:
    _GORDER.append(_hi); _hi -= 1
    if _lo <= _hi:
        _GORDER.append(_lo); _lo += 1


def _build_program(reps=1):
    nc = bacc.Bacc("TRN2", target_bir_lowering=False, debug=False)

    xT = nc.dram_tensor("xT", [D, HALF], BF16, kind="ExternalInput").ap()
    xTq = nc.dram_tensor("xTq", [D, S], BF16, kind="ExternalInput").ap()
    wq = nc.dram_tensor("wq", [D, D], BF16, kind="ExternalInput").ap()
    wk = nc.dram_tensor("wk", [D, D], BF16, kind="ExternalInput").ap()
    wv = nc.dram_tensor("wv", [D, D], BF16, kind="ExternalInput").ap()
    mask = nc.dram_tensor("mask", [NQB, P, 256], F32, kind="ExternalInput").ap()
    y = nc.dram_tensor("y", [S, D], F32, kind="ExternalOutput").ap()
    ell = nc.dram_tensor("ell", [P, NQB], F32, kind="ExternalOutput").ap()

    with tile.TileContext(nc) as tc:
        with (
            tc.tile_pool(name="big", bufs=1) as big,
            tc.tile_pool(name="wpool", bufs=2) as wpool,
            tc.tile_pool(name="xslab", bufs=2) as xslab,
            tc.tile_pool(name="mk", bufs=2) as mk_pool,
            tc.tile_pool(name="pp", bufs=4) as pp_pool,
            tc.tile_pool(name="pT", bufs=3) as pT_pool,
            tc.tile_pool(name="yy", bufs=2) as y_pool,
            tc.tile_pool(name="st", bufs=2) as st_pool,
            tc.tile_pool(name="ps", bufs=6, space="PSUM") as ps,
            tc.tile_pool(name="yp", bufs=2, space="PSUM") as yp_pool,
        ):
          for _rep in range(reps):
            KT = big.tile([P, DC, HALF], BF16, tag="KT")     # 32 KB/part
            V = big.tile([P, 16, D], BF16, tag="V")          # 32 KB/part
            QT = big.tile([P, DC, S], BF16, tag="QT")        # 64 KB/part
            ells_all = big.tile([P, NQB], F32, tag="ells_all")
            ident = big.tile([P, P], BF16, tag="ident")
            make_identity(nc, ident[:])

            # ---- fused K^T + V projection over the local key half ------
            wk_t = wpool.tile([P, DC, D], BF16, tag="W")
            for _i in range(DC):
                nc.scalar.dma_start(
                    out=wk_t[:, _i, :], in_=wk[_i * P : (_i + 1) * P, :]
                )
            wv_t = wpool.tile([P, DC, D], BF16, tag="W")
            for _i in range(DC):
                nc.scalar.dma_start(
                    out=wv_t[:, _i, :], in_=wv[_i * P : (_i + 1) * P, :]
                )
            for kt in range(4):  # local key tiles of 512
                xs = xslab.tile([P, DC, 512], BF16, tag="xs")
                for _i in range(DC):
                    nc.sync.dma_start(
                        out=xs[:, _i, :],
                        in_=xT[_i * P : (_i + 1) * P,
                               kt * 512 : (kt + 1) * 512],
                    )
                for j in range(DC):
                    pt = ps.tile([P, 512], F32, tag="ps", name=f"kp{kt}_{j}")
                    for i in range(DC):
                        nc.tensor.matmul(
                            pt[:],
                            lhsT=wk_t[:, i, j * P : (j + 1) * P],
                            rhs=xs[:, i, :],
                            start=(i == 0),
                            stop=(i == DC - 1),
                        )
                    nc.vector.tensor_copy(
                        KT[:, j, kt * 512 : (kt + 1) * 512], pt[:]
                    )
                for sb in range(4):
                    kb = kt * 4 + sb
                    pv = [ps.tile([P, 512], F32, tag="ps", name=f"v{n}_{kb}")
                          for n in range(2)]
                    for i in range(DC):
                        for n in range(2):
                            nc.tensor.matmul(
                                pv[n][:],
                                lhsT=xs[:, i, sb * P : (sb + 1) * P],
                                rhs=wv_t[:, i, n * 512 : (n + 1) * 512],
                                start=(i == 0),
                                stop=(i == DC - 1),
                            )
                    for n in range(2):
                        nc.scalar.copy(V[:, kb, n * 512 : (n + 1) * 512], pv[n][:])

            # ---- Q^T projection for ALL queries, SBUF-resident ---------
            wq_t = wpool.tile([P, DC, D], BF16, tag="W")
            for _i in range(DC):
                nc.scalar.dma_start(
                    out=wq_t[:, _i, :], in_=wq[_i * P : (_i + 1) * P, :]
                )
            for qt in range(8):  # q tiles of 512
                xs = xslab.tile([P, DC, 512], BF16, tag="xs")
                for _i in range(DC):
                    nc.sync.dma_start(
                        out=xs[:, _i, :],
                        in_=xTq[_i * P : (_i + 1) * P,
                                qt * 512 : (qt + 1) * 512],
                    )
                for j in range(DC):
                    pt = ps.tile([P, 512], F32, tag="ps", name=f"qp{qt}_{j}")
                    for i in range(DC):
                        nc.tensor.matmul(
                            pt[:],
                            lhsT=wq_t[:, i, j * P : (j + 1) * P],
                            rhs=xs[:, i, :],
                            start=(i == 0),
                            stop=(i == DC - 1),
                        )
                    nc.vector.tensor_copy(
                        QT[:, j, qt * 512 : (qt + 1) * 512], pt[:]
                    )

            # ---- partial causal attention over the local keyset --------
            for g in _GORDER:
                L = _LS[g]
                cols = L * P
                T = (cols + 511) // 512
                widths = [512] * (T - 1) + [cols - 512 * (T - 1)]
                mw = 128 if L == 1 else 256

                mk = mk_pool.tile([P, 256], F32, tag="mk")
                nc.sync.dma_start(out=mk[:], in_=mask[g])

                pts = []
                for t in range(T):
                    pts.append(ps.tile([P, widths[t]], F32, tag="ps",
                                       name=f"sc{g}_{t}"))
                for i in range(DC):
                    for t in range(T):
                        nc.tensor.matmul(
                            pts[t][:],
                            lhsT=QT[:, i, g * P : (g + 1) * P],
                            rhs=KT[:, i, t * 512 : t * 512 + widths[t]],
                            start=(i == 0),
                            stop=(i == DC - 1),
                        )

                # additive mask on the last mw local kv columns (the
                # window can straddle the last two PSUM tiles)
                rem = mw
                moff = mw
                ti = T - 1
                while rem > 0:
                    w = widths[ti]
                    take = min(rem, w)
                    nc.vector.tensor_add(
                        pts[ti][:, w - take : w],
                        pts[ti][:, w - take : w],
                        mk[:, moff - take : moff],
                    )
                    rem -= take
                    moff -= take
                    ti -= 1

                ells = st_pool.tile([P, 8], F32, tag="ells")
                yps = [yp_pool.tile([P, 512], F32, tag="yp", name=f"y{n}_{g}")
                       for n in range(2)]

                def attnv(m, psb):
                    for n in range(2):
                        nc.tensor.matmul(
                            yps[n][:],
                            lhsT=psb[:],
                            rhs=V[:, m, n * 512 : (n + 1) * 512],
                            start=(m == 0),
                            stop=(m == L - 1),
                        )

                kc = 0
                pending = None
                for t in range(T):
                    ppt = pp_pool.tile([P, widths[t]], BF16, tag="pp",
                                       name=f"pp{g}_{t}")
                    nc.scalar.activation(
                        ppt[:],
                        pts[t][:],
                        mybir.ActivationFunctionType.Exp,
                        bias=0.0,
                        scale=SCALE,
                        accum_out=ells[:, t : t + 1],
                    )
                    for cch in range(widths[t] // P):
                        ptp = ps.tile([P, P], BF16, tag="ps", name=f"tp{g}_{kc}")
                        nc.tensor.transpose(
                            ptp[:], ppt[:, cch * P : (cch + 1) * P], ident[:]
                        )
                        psb = pT_pool.tile([P, P], BF16, tag="pT",
                                           name=f"pb{g}_{kc}")
                        nc.vector.tensor_copy(psb[:], ptp[:])
                        if pending is not None:
                            attnv(*pending)
                        pending = (kc, psb)
                        kc += 1
                attnv(*pending)

                nc.vector.tensor_reduce(
                    ells_all[:, g : g + 1],
                    ells[:, :T],
                    axis=mybir.AxisListType.X,
                    op=mybir.AluOpType.add,
                )

                for n in range(2):  # unnormalized numerator out
                    ys = y_pool.tile([P, 512], F32, tag="y")
                    nc.scalar.copy(ys[:], yps[n][:])
                    nc.sync.dma_start(
                        out=y[g * P : (g + 1) * P, n * 512 : (n + 1) * 512],
                        in_=ys[:],
                    )

            nc.sync.dma_start(out=ell[:, :], in_=ells_all[:])
    nc.finalize()
    return nc


_NC = None


def _get_program():
    global _NC
    if _NC is None:
        _NC = _build_program()
    return _NC


def _build_mask(c):
    """mask[g, :, j] (j < mw) applies to local kv col L*128 - mw + j."""
    ks = _keyset(c)
    m = np.full((NQB, P, 256), NEG, np.float32)
    for g in range(NQB):
        L = _LS[g]
        cnt = sum(1 for b in ks if b <= g)
        mw = 128 if L == 1 else 256
        q = g * P + np.arange(P)[:, None]
        for wi in range(mw // P):
            mlocal = L - mw // P + wi
            if mlocal < cnt:
                gb = ks[mlocal]
                k = gb * P + np.arange(P)[None, :]
                m[g, :, wi * P : (wi + 1) * P] = np.where(k <= q, 0.0, NEG)
    return m


def _make_in_maps(x, Wq, Wk, Wv):
    bf = ml_dtypes.bfloat16
    wqb = np.ascontiguousarray(Wq.astype(bf))
    wkb = np.ascontiguousarray(Wk.astype(bf))
    wvb = np.ascontiguousarray(Wv.astype(bf))
    masks = [_build_mask(0), _build_mask(1)]
    keycols = [
        np.concatenate([np.arange(b * P, (b + 1) * P) for b in _keyset(c)])
        for c in (0, 1)
    ]

    in_maps = []
    for core in range(8):
        b, c = core // 2, core % 2
        xb = x[b]
        in_maps.append(
            {
                "xT": np.ascontiguousarray(xb[keycols[c]].T.astype(bf)),
                "xTq": np.ascontiguousarray(xb.T.astype(bf)),
                "wq": wqb,
                "wk": wkb,
                "wv": wvb,
                "mask": masks[c],
            }
        )
    return in_maps


def _bench_in_maps():
    rng = np.random.default_rng(0)
    s = 1.0 / math.sqrt(D)
    x = rng.standard_normal((B, S, D)).astype(np.float32)
    Wq = rng.uniform(-s, s, (D, D)).astype(np.float32)
    Wk = rng.uniform(-s, s, (D, D)).astype(np.float32)
    Wv = rng.uniform(-s, s, (D, D)).astype(np.float32)
    return _make_in_maps(x, Wq, Wk, Wv)


def kernel(x, Wq, Wk, Wv):
    nc = _get_program()
    in_maps = _make_in_maps(x, Wq, Wk, Wv)
    res = run_bass_kernel_spmd(nc, in_maps, core_ids=list(range(8))).results

    out = np.empty((B, S, D), np.float32)
    for b in range(B):
        r0, r1 = res[2 * b], res[2 * b + 1]
        num = r0["y"] + r1["y"]
        # ell[p, g] -> per-row: q = g*128 + p
        l0 = r0["ell"].T.reshape(S, 1)
        l1 = r1["ell"].T.reshape(S, 1)
        out[b] = num / (l0 + l1)
    return out



# revision 3
# speedup vs baseline: 1.2331x; 1.2331x over previous
"""Causal attention on 8 TRN2 cores — key-sharded, transpose-free variant.

2 cores per batch; the pair splits the KEYS: parity c owns key blocks
S_c = sorted({2p+c} u {31-2p-c}), projects K^T/V for only its 2048 keys
plus Q^T for all 4096 queries, then computes partial unnormalized
attention over its keyset for every query. Host combines
y = (num0+num1)/(ell0+ell1) — exact, since exp uses no max shift.

Unlike v1, scores are computed TRANSPOSED (S^T[k, q] = K_blk^T . Q) so the
exp'd probabilities feed attnv matmuls directly as lhsT — no PE-mode
transposes (~275ns each on HW) and no PSUM->SBUF P-block copies. Queries
are processed in 16 chunks of 256; the causal boundary needs only two
constant [128, 256] additive masks (picked per chunk half by parity via
the cmask input). Row sums (ell) come from a free-dim-1 matmul against a
ones vector, accumulated in PSUM alongside the numerators.
"""

import math
import sys

sys.path.insert(0, "/opt/trn_rl_repo")

import ml_dtypes
import numpy as np

import concourse.mybir as mybir
import concourse.tile as tile
from concourse import bacc
from concourse.bass_utils import run_bass_kernel_spmd

B = 4
S = 4096
D = 1024
P = 128
DC = D // P
NKB = 32             # global key blocks per batch
NQB = 32             # q blocks (128 queries each)
NCH = 16             # q chunks (256 queries each)
HALF = S // 2        # keys owned per core
BF16 = mybir.dt.bfloat16
F32 = mybir.dt.float32
NEG = -1.0e9
SCALE = 1.0 / math.sqrt(D)


def _keyset(c):
    return sorted({2 * p + c for p in range(8)} | {31 - 2 * p - c for p in range(8)})


def _build_program(reps=1):
    nc = bacc.Bacc("TRN2", target_bir_lowering=False, debug=False)

    xT = nc.dram_tensor("xT", [D, HALF], BF16, kind="ExternalInput").ap()
    xTq = nc.dram_tensor("xTq", [D, S], BF16, kind="ExternalInput").ap()
    wq = nc.dram_tensor("wq", [D, D], BF16, kind="ExternalInput").ap()
    wk = nc.dram_tensor("wk", [D, D], BF16, kind="ExternalInput").ap()
    wv = nc.dram_tensor("wv", [D, D], BF16, kind="ExternalInput").ap()
    cmask = nc.dram_tensor("cmask", [2, P, 256], F32, kind="ExternalInput").ap()
    y = nc.dram_tensor("y", [S, D], F32, kind="ExternalOutput").ap()
    ell = nc.dram_tensor("ell", [P, NQB], F32, kind="ExternalOutput").ap()

    with tile.TileContext(nc) as tc:
        with (
            tc.tile_pool(name="big", bufs=1) as big,
            tc.tile_pool(name="wpool", bufs=2) as wpool,
            tc.tile_pool(name="xslab", bufs=2) as xslab,
            tc.tile_pool(name="pp", bufs=3) as pp_pool,
            tc.tile_pool(name="ydr", bufs=4) as ydr_pool,
            tc.tile_pool(name="yp", bufs=4, space="PSUM") as yp_pool,
            tc.tile_pool(name="scp", bufs=2, space="PSUM") as sc_pool,
            tc.tile_pool(name="elp", bufs=2, space="PSUM") as el_pool,
        ):
          for _rep in range(reps):
            KT = big.tile([P, DC, HALF], BF16, tag="KT")     # 32 KB/part
            V = big.tile([P, 16, D], BF16, tag="V")          # 32 KB/part
            QT = big.tile([P, DC, S], BF16, tag="QT")        # 64 KB/part
            ells_all = big.tile([P, NQB], F32, tag="ells_all")
            ones = big.tile([P, 1], BF16, tag="ones")
            cm = big.tile([P, 2, 256], F32, tag="cm")
            nc.gpsimd.memset(ones[:], 1.0)
            for s in range(2):
                nc.sync.dma_start(out=cm[:, s, :], in_=cmask[s])

            # ---- fused K^T + V projection over the local key half ------
            wk_t = wpool.tile([P, DC, D], BF16, tag="W")
            for _i in range(DC):
                nc.scalar.dma_start(
                    out=wk_t[:, _i, :], in_=wk[_i * P : (_i + 1) * P, :]
                )
            wv_t = wpool.tile([P, DC, D], BF16, tag="W")
            for _i in range(DC):
                nc.scalar.dma_start(
                    out=wv_t[:, _i, :], in_=wv[_i * P : (_i + 1) * P, :]
                )
            for kt in range(4):  # local key tiles of 512
                xs = xslab.tile([P, DC, 512], BF16, tag="xs")
                for _i in range(DC):
                    nc.sync.dma_start(
                        out=xs[:, _i, :],
                        in_=xT[_i * P : (_i + 1) * P,
                               kt * 512 : (kt + 1) * 512],
                    )
                for j in range(DC):
                    pt = yp_pool.tile([P, 512], F32, tag="yp", name=f"kp{kt}_{j}")
                    for i in range(DC):
                        nc.tensor.matmul(
                            pt[:],
                            lhsT=wk_t[:, i, j * P : (j + 1) * P],
                            rhs=xs[:, i, :],
                            start=(i == 0),
                            stop=(i == DC - 1),
                        )
                    nc.vector.tensor_copy(
                        KT[:, j, kt * 512 : (kt + 1) * 512], pt[:]
                    )
                for sb in range(4):
                    kb = kt * 4 + sb
                    pv = [yp_pool.tile([P, 512], F32, tag="yp", name=f"v{n}_{kb}")
                          for n in range(2)]
                    for i in range(DC):
                        for n in range(2):
                            nc.tensor.matmul(
                                pv[n][:],
                                lhsT=xs[:, i, sb * P : (sb + 1) * P],
                                rhs=wv_t[:, i, n * 512 : (n + 1) * 512],
                                start=(i == 0),
                                stop=(i == DC - 1),
                            )
                    for n in range(2):
                        nc.scalar.copy(V[:, kb, n * 512 : (n + 1) * 512], pv[n][:])

            # ---- Q^T projection for ALL queries, SBUF-resident ---------
            wq_t = wpool.tile([P, DC, D], BF16, tag="W")
            for _i in range(DC):
                nc.scalar.dma_start(
                    out=wq_t[:, _i, :], in_=wq[_i * P : (_i + 1) * P, :]
                )
            for qt in range(8):  # q tiles of 512
                xs = xslab.tile([P, DC, 512], BF16, tag="xs")
                for _i in range(DC):
                    nc.sync.dma_start(
                        out=xs[:, _i, :],
                        in_=xTq[_i * P : (_i + 1) * P,
                                qt * 512 : (qt + 1) * 512],
                    )
                for j in range(DC):
                    pt = yp_pool.tile([P, 512], F32, tag="yp", name=f"qp{qt}_{j}")
                    for i in range(DC):
                        nc.tensor.matmul(
                            pt[:],
                            lhsT=wq_t[:, i, j * P : (j + 1) * P],
                            rhs=xs[:, i, :],
                            start=(i == 0),
                            stop=(i == DC - 1),
                        )
                    nc.vector.tensor_copy(
                        QT[:, j, qt * 512 : (qt + 1) * 512], pt[:]
                    )

            # ---- partial causal attention, S^T layout ------------------
            # chunk c covers q blocks {2c, 2c+1}; active local key blocks
            # are j = 0..c (A_c = c+1 on both parities). Block j == c is
            # the causal-partial one; cm[:, c//8, :] masks it.
            state = {}  # per-chunk live tiles: yps, els

            def emit_attnv(c, j, ppt):
                if j == 0:
                    state["yps"] = [
                        [yp_pool.tile([P, 512], F32, tag="yp",
                                      name=f"ya{c}_{h}{n}")
                         for n in range(2)]
                        for h in range(2)
                    ]
                    state["els"] = [
                        el_pool.tile([P, 1], F32, tag="el", name=f"el{c}_{h}")
                        for h in range(2)
                    ]
                yps, els = state["yps"], state["els"]
                first, last = (j == 0), (j == c)
                for h in range(2):
                    lh = ppt[:, h * P : (h + 1) * P]
                    for n in range(2):
                        nc.tensor.matmul(
                            yps[h][n][:],
                            lhsT=lh,
                            rhs=V[:, j, n * 512 : (n + 1) * 512],
                            start=first,
                            stop=last,
                        )
                    nc.tensor.matmul(
                        els[h][:], lhsT=lh, rhs=ones[:], start=first, stop=last
                    )
                if last:
                    for h in range(2):
                        g = 2 * c + h
                        nc.vector.tensor_copy(
                            ells_all[:, g : g + 1], els[h][:]
                        )
                        for n in range(2):
                            ys = ydr_pool.tile([P, 512], F32, tag="ydr",
                                               name=f"yd{c}_{h}{n}")
                            nc.vector.tensor_copy(ys[:], yps[h][n][:])
                            nc.sync.dma_start(
                                out=y[g * P : (g + 1) * P,
                                      n * 512 : (n + 1) * 512],
                                in_=ys[:],
                            )

            pending = None
            for c in range(NCH):
                qc0 = c * 256
                for j in range(c + 1):
                    st = sc_pool.tile([P, 256], F32, tag="sc",
                                      name=f"sc{c}_{j}")
                    for i in range(DC):
                        nc.tensor.matmul(
                            st[:],
                            lhsT=KT[:, i, j * P : (j + 1) * P],
                            rhs=QT[:, i, qc0 : qc0 + 256],
                            start=(i == 0),
                            stop=(i == DC - 1),
                        )
                    if j == c:
                        nc.vector.tensor_add(st[:], st[:], cm[:, c // 8, :])
                    ppt = pp_pool.tile([P, 256], BF16, tag="pp",
                                       name=f"pp{c}_{j}")
                    nc.scalar.activation(
                        ppt[:],
                        st[:],
                        mybir.ActivationFunctionType.Exp,
                        bias=0.0,
                        scale=SCALE,
                    )
                    if pending is not None:
                        emit_attnv(*pending)
                    pending = (c, j, ppt)
            emit_attnv(*pending)

            nc.sync.dma_start(out=ell[:, :], in_=ells_all[:])
    nc.finalize()
    return nc


_NC = None


def _get_program():
    global _NC
    if _NC is None:
        _NC = _build_program()
    return _NC


def _build_cmasks(c):
    """cmask[s] is the additive mask for the causal-partial key block of
    chunks [8s, 8s+8); its key-block offset within the chunk depends on
    core parity c."""

    def mask_for_offset(o):
        m = np.zeros((P, 256), np.float32)
        p = np.arange(P)[:, None]
        j = np.arange(P)[None, :]
        tri = np.where(p <= j, 0.0, NEG).astype(np.float32)
        if o == 0:
            m[:, :P] = tri
        else:
            m[:, :P] = NEG
            m[:, P:] = tri
        return m

    return np.stack([mask_for_offset(c), mask_for_offset(1 - c)])


def _make_in_maps(x, Wq, Wk, Wv):
    bf = ml_dtypes.bfloat16
    wqb = np.ascontiguousarray(Wq.astype(bf))
    wkb = np.ascontiguousarray(Wk.astype(bf))
    wvb = np.ascontiguousarray(Wv.astype(bf))
    cmasks = [_build_cmasks(0), _build_cmasks(1)]
    keycols = [
        np.concatenate([np.arange(b * P, (b + 1) * P) for b in _keyset(c)])
        for c in (0, 1)
    ]

    in_maps = []
    for core in range(8):
        b, c = core // 2, core % 2
        xb = x[b]
        in_maps.append(
            {
                "xT": np.ascontiguousarray(xb[keycols[c]].T.astype(bf)),
                "xTq": np.ascontiguousarray(xb.T.astype(bf)),
                "wq": wqb,
                "wk": wkb,
                "wv": wvb,
                "cmask": cmasks[c],
            }
        )
    return in_maps


def _bench_in_maps():
    rng = np.random.default_rng(0)
    s = 1.0 / math.sqrt(D)
    x = rng.standard_normal((B, S, D)).astype(np.float32)
    Wq = rng.uniform(-s, s, (D, D)).astype(np.float32)
    Wk = rng.uniform(-s, s, (D, D)).astype(np.float32)
    Wv = rng.uniform(-s, s, (D, D)).astype(np.float32)
    return _make_in_maps(x, Wq, Wk, Wv)


def kernel(x, Wq, Wk, Wv):
    nc = _get_program()
    in_maps = _make_in_maps(x, Wq, Wk, Wv)
    res = run_bass_kernel_spmd(nc, in_maps, core_ids=list(range(8))).results

    out = np.empty((B, S, D), np.float32)
    for b in range(B):
        r0, r1 = res[2 * b], res[2 * b + 1]
        num = r0["y"] + r1["y"]
        # ell[p, g] -> per-row: q = g*128 + p
        l0 = r0["ell"].T.reshape(S, 1)
        l1 = r1["ell"].T.reshape(S, 1)
        out[b] = num / (l0 + l1)
    return out


# revision 8
# speedup vs baseline: 1.4077x; 1.1416x over previous
"""Causal attention on 8 TRN2 cores — key-sharded, transpose-free variant.

2 cores per batch; the pair splits the KEYS: parity c owns key blocks
S_c = sorted({2p+c} u {31-2p-c}), projects K^T/V for only its 2048 keys
plus Q^T for all 4096 queries, then computes partial unnormalized
attention over its keyset for every query. Host combines
y = (num0+num1)/(ell0+ell1) — exact, since exp uses no max shift.

Unlike v1, scores are computed TRANSPOSED (S^T[k, q] = K_blk^T . Q) so the
exp'd probabilities feed attnv matmuls directly as lhsT — no PE-mode
transposes (~275ns each on HW) and no PSUM->SBUF P-block copies. Queries
are processed in 16 chunks of 256; the causal boundary needs only two
constant [128, 256] additive masks (picked per chunk half by parity via
the cmask input). Row sums (ell) come from a free-dim-1 matmul against a
ones vector, accumulated in PSUM alongside the numerators.
"""

import math
import sys

sys.path.insert(0, "/opt/trn_rl_repo")

import ml_dtypes
import numpy as np

import concourse.mybir as mybir
import concourse.tile as tile
from concourse import bacc
from concourse.bass_utils import run_bass_kernel_spmd

B = 4
S = 4096
D = 1024
P = 128
DC = D // P
NKB = 32             # global key blocks per batch
NQB = 32             # q blocks (128 queries each)
NCH = 16             # q chunks (256 queries each)
HALF = S // 2        # keys owned per core
BF16 = mybir.dt.bfloat16
F32 = mybir.dt.float32
NEG = -1.0e9
SCALE = 1.0 / math.sqrt(D)


def _keyset(c):
    return sorted({2 * p + c for p in range(8)} | {31 - 2 * p - c for p in range(8)})


def _build_program(reps=1):
    nc = bacc.Bacc("TRN2", target_bir_lowering=False, debug=False, num_devices=8)

    xT = nc.dram_tensor("xT", [D, HALF], BF16, kind="ExternalInput").ap()
    # local query half: parity 0 owns queries [0, 2048), parity 1 [2048, 4096)
    xTq = nc.dram_tensor("xTq", [D, HALF], BF16, kind="ExternalInput").ap()
    wq = nc.dram_tensor("wq", [D, D], BF16, kind="ExternalInput").ap()
    wk = nc.dram_tensor("wk", [D, D], BF16, kind="ExternalInput").ap()
    wv = nc.dram_tensor("wv", [D, D], BF16, kind="ExternalInput").ap()
    cmask = nc.dram_tensor("cmask", [2, P, 256], F32, kind="ExternalInput").ap()
    y = nc.dram_tensor("y", [S, D], F32, kind="ExternalOutput").ap()
    ell = nc.dram_tensor("ell", [P, NQB], F32, kind="ExternalOutput").ap()
    # pair Q^T exchange: each core projects its query half, AllGathers the
    # pair's halves (rank order puts queries [0,2048) at index 0), reads
    # the full Q^T back — no duplicated Q projection.
    qloc = nc.dram_tensor("qloc", [D, HALF], BF16)
    qfull = nc.dram_tensor("qfull", [2, D, HALF], BF16)

    with tile.TileContext(nc) as tc:
        with (
            tc.tile_pool(name="big", bufs=1) as big,
            tc.tile_pool(name="wpool", bufs=2) as wpool,
            tc.tile_pool(name="xslab", bufs=2) as xslab,
            tc.tile_pool(name="pp", bufs=3) as pp_pool,
            tc.tile_pool(name="ydr", bufs=4) as ydr_pool,
            tc.tile_pool(name="qdr", bufs=3) as qdr_pool,
            tc.tile_pool(name="yp", bufs=4, space="PSUM") as yp_pool,
            tc.tile_pool(name="scp", bufs=2, space="PSUM") as sc_pool,
            tc.tile_pool(name="elp", bufs=2, space="PSUM") as el_pool,
        ):
          for _rep in range(reps):
            KT = big.tile([P, DC, HALF], BF16, tag="KT")     # 32 KB/part
            V = big.tile([P, 16, D], BF16, tag="V")          # 32 KB/part
            QT = big.tile([P, DC, S], BF16, tag="QT")        # 64 KB/part
            ells_all = big.tile([P, NQB], F32, tag="ells_all")
            ones = big.tile([P, 1], BF16, tag="ones")
            cm = big.tile([P, 2, 256], F32, tag="cm")
            nc.gpsimd.memset(ones[:], 1.0)
            for s in range(2):
                nc.sync.dma_start(out=cm[:, s, :], in_=cmask[s])

            # ---- Q^T projection for the LOCAL query half ---------------
            wq_t = wpool.tile([P, DC, D], BF16, tag="W")
            for _i in range(DC):
                nc.scalar.dma_start(
                    out=wq_t[:, _i, :], in_=wq[_i * P : (_i + 1) * P, :]
                )
            for qt in range(4):  # local q tiles of 512
                xs = xslab.tile([P, DC, 512], BF16, tag="xs")
                for _i in range(DC):
                    nc.sync.dma_start(
                        out=xs[:, _i, :],
                        in_=xTq[_i * P : (_i + 1) * P,
                                qt * 512 : (qt + 1) * 512],
                    )
                for j in range(DC):
                    pt = yp_pool.tile([P, 512], F32, tag="yp", name=f"qp{qt}_{j}")
                    for i in range(DC):
                        nc.tensor.matmul(
                            pt[:],
                            lhsT=wq_t[:, i, j * P : (j + 1) * P],
                            rhs=xs[:, i, :],
                            start=(i == 0),
                            stop=(i == DC - 1),
                        )
                    qd = qdr_pool.tile([P, 512], BF16, tag="qd",
                                       name=f"qd{qt}_{j}")
                    nc.vector.tensor_copy(qd[:], pt[:])
                    nc.sync.dma_start(
                        out=qloc[j * P : (j + 1) * P,
                                 qt * 512 : (qt + 1) * 512],
                        in_=qd[:],
                    )
            nc.gpsimd.collective_compute(
                "AllGather",
                mybir.AluOpType.bypass,
                replica_groups=[[2 * b, 2 * b + 1] for b in range(4)],
                ins=[qloc[:, :]],
                outs=[qfull[:, :, :]],
            )

            # ---- fused K^T + V projection over the local key half ------
            wk_t = wpool.tile([P, DC, D], BF16, tag="W")
            for _i in range(DC):
                nc.scalar.dma_start(
                    out=wk_t[:, _i, :], in_=wk[_i * P : (_i + 1) * P, :]
                )
            wv_t = wpool.tile([P, DC, D], BF16, tag="W")
            for _i in range(DC):
                nc.scalar.dma_start(
                    out=wv_t[:, _i, :], in_=wv[_i * P : (_i + 1) * P, :]
                )
            for kt in range(4):  # local key tiles of 512
                xs = xslab.tile([P, DC, 512], BF16, tag="xs")
                for _i in range(DC):
                    nc.sync.dma_start(
                        out=xs[:, _i, :],
                        in_=xT[_i * P : (_i + 1) * P,
                               kt * 512 : (kt + 1) * 512],
                    )
                for j in range(DC):
                    pt = yp_pool.tile([P, 512], F32, tag="yp", name=f"kp{kt}_{j}")
                    for i in range(DC):
                        nc.tensor.matmul(
                            pt[:],
                            lhsT=wk_t[:, i, j * P : (j + 1) * P],
                            rhs=xs[:, i, :],
                            start=(i == 0),
                            stop=(i == DC - 1),
                        )
                    nc.vector.tensor_copy(
                        KT[:, j, kt * 512 : (kt + 1) * 512], pt[:]
                    )
                for sb in range(4):
                    kb = kt * 4 + sb
                    pv = [yp_pool.tile([P, 512], F32, tag="yp", name=f"v{n}_{kb}")
                          for n in range(2)]
                    for i in range(DC):
                        for n in range(2):
                            nc.tensor.matmul(
                                pv[n][:],
                                lhsT=xs[:, i, sb * P : (sb + 1) * P],
                                rhs=wv_t[:, i, n * 512 : (n + 1) * 512],
                                start=(i == 0),
                                stop=(i == DC - 1),
                            )
                    for n in range(2):
                        nc.scalar.copy(V[:, kb, n * 512 : (n + 1) * 512], pv[n][:])

            # ---- read back the pair's full Q^T ------------------------
            for i in range(DC):
                for hf in range(2):
                    nc.sync.dma_start(
                        out=QT[:, i, hf * HALF : (hf + 1) * HALF],
                        in_=qfull[hf, i * P : (i + 1) * P, :],
                    )

            # ---- partial causal attention, S^T layout ------------------
            # chunk c covers q blocks {2c, 2c+1}; active local key blocks
            # are j = 0..c (A_c = c+1 on both parities). Block j == c is
            # the causal-partial one; cm[:, c//8, :] masks it.
            state = {}  # per-chunk live tiles: yps, els

            def emit_attnv(c, j, ppt):
                if j == 0:
                    state["yps"] = [
                        [yp_pool.tile([P, 512], F32, tag="yp",
                                      name=f"ya{c}_{h}{n}")
                         for n in range(2)]
                        for h in range(2)
                    ]
                    state["els"] = [
                        el_pool.tile([P, 1], F32, tag="el", name=f"el{c}_{h}")
                        for h in range(2)
                    ]
                yps, els = state["yps"], state["els"]
                first, last = (j == 0), (j == c)
                for h in range(2):
                    lh = ppt[:, h * P : (h + 1) * P]
                    for n in range(2):
                        nc.tensor.matmul(
                            yps[h][n][:],
                            lhsT=lh,
                            rhs=V[:, j, n * 512 : (n + 1) * 512],
                            start=first,
                            stop=last,
                        )
                    nc.tensor.matmul(
                        els[h][:], lhsT=lh, rhs=ones[:], start=first, stop=last
                    )
                if last:
                    for h in range(2):
                        g = 2 * c + h
                        nc.vector.tensor_copy(
                            ells_all[:, g : g + 1], els[h][:]
                        )
                        for n in range(2):
                            ys = ydr_pool.tile([P, 512], F32, tag="ydr",
                                               name=f"yd{c}_{h}{n}")
                            nc.vector.tensor_copy(ys[:], yps[h][n][:])
                            nc.sync.dma_start(
                                out=y[g * P : (g + 1) * P,
                                      n * 512 : (n + 1) * 512],
                                in_=ys[:],
                            )

            pending = None
            for c in range(NCH):
                qc0 = c * 256
                for j in range(c + 1):
                    st = sc_pool.tile([P, 256], F32, tag="sc",
                                      name=f"sc{c}_{j}")
                    for i in range(DC):
                        nc.tensor.matmul(
                            st[:],
                            lhsT=KT[:, i, j * P : (j + 1) * P],
                            rhs=QT[:, i, qc0 : qc0 + 256],
                            start=(i == 0),
                            stop=(i == DC - 1),
                        )
                    if j == c:
                        nc.vector.tensor_add(st[:], st[:], cm[:, c // 8, :])
                    ppt = pp_pool.tile([P, 256], BF16, tag="pp",
                                       name=f"pp{c}_{j}")
                    nc.scalar.activation(
                        ppt[:],
                        st[:],
                        mybir.ActivationFunctionType.Exp,
                        bias=0.0,
                        scale=SCALE,
                    )
                    if pending is not None:
                        emit_attnv(*pending)
                    pending = (c, j, ppt)
            emit_attnv(*pending)

            nc.sync.dma_start(out=ell[:, :], in_=ells_all[:])
    nc.finalize()
    return nc


_NC = None


def _get_program():
    global _NC
    if _NC is None:
        _NC = _build_program()
    return _NC


def _build_cmasks(c):
    """cmask[s] is the additive mask for the causal-partial key block of
    chunks [8s, 8s+8); its key-block offset within the chunk depends on
    core parity c."""

    def mask_for_offset(o):
        m = np.zeros((P, 256), np.float32)
        p = np.arange(P)[:, None]
        j = np.arange(P)[None, :]
        tri = np.where(p <= j, 0.0, NEG).astype(np.float32)
        if o == 0:
            m[:, :P] = tri
        else:
            m[:, :P] = NEG
            m[:, P:] = tri
        return m

    return np.stack([mask_for_offset(c), mask_for_offset(1 - c)])


def _make_in_maps(x, Wq, Wk, Wv):
    bf = ml_dtypes.bfloat16
    wqb = np.ascontiguousarray(Wq.astype(bf))
    wkb = np.ascontiguousarray(Wk.astype(bf))
    wvb = np.ascontiguousarray(Wv.astype(bf))
    cmasks = [_build_cmasks(0), _build_cmasks(1)]
    keycols = [
        np.concatenate([np.arange(b * P, (b + 1) * P) for b in _keyset(c)])
        for c in (0, 1)
    ]

    in_maps = []
    for core in range(8):
        b, c = core // 2, core % 2
        xb = x[b]
        in_maps.append(
            {
                "xT": np.ascontiguousarray(xb[keycols[c]].T.astype(bf)),
                "xTq": np.ascontiguousarray(
                    xb[c * HALF : (c + 1) * HALF].T.astype(bf)
                ),
                "wq": wqb,
                "wk": wkb,
                "wv": wvb,
                "cmask": cmasks[c],
            }
        )
    return in_maps


def _bench_in_maps():
    rng = np.random.default_rng(0)
    s = 1.0 / math.sqrt(D)
    x = rng.standard_normal((B, S, D)).astype(np.float32)
    Wq = rng.uniform(-s, s, (D, D)).astype(np.float32)
    Wk = rng.uniform(-s, s, (D, D)).astype(np.float32)
    Wv = rng.uniform(-s, s, (D, D)).astype(np.float32)
    return _make_in_maps(x, Wq, Wk, Wv)


def kernel(x, Wq, Wk, Wv):
    nc = _get_program()
    in_maps = _make_in_maps(x, Wq, Wk, Wv)
    res = run_bass_kernel_spmd(nc, in_maps, core_ids=list(range(8))).results

    out = np.empty((B, S, D), np.float32)
    for b in range(B):
        r0, r1 = res[2 * b], res[2 * b + 1]
        num = r0["y"] + r1["y"]
        # ell[p, g] -> per-row: q = g*128 + p
        l0 = r0["ell"].T.reshape(S, 1)
        l1 = r1["ell"].T.reshape(S, 1)
        out[b] = num / (l0 + l1)
    return out
